# revision 7
# baseline (speedup 1.0000x reference)
"""Trainium2 Bass kernel for nn_DeepFM_27169963114981.

Strategy
--------
The reference DeepFM (eval mode, affine BN, no nonlinearity in the deep MLP)
collapses algebraically:

  * deep MLP output-sum is LINEAR in its input  -> fold into a 624-vector,
    then fold per-field slices of that vector into the embedding tables.
  * fm_first / fm_second / deep contributions of every field (dense AND
    sparse -- the "dense" feature values are integers in [0, V)) become a
    single unified table lookup of 19 floats per (sample, field):
        row = [ vec16 | c16 | c17 | c18 ]
    combined per sample with per-(sample,field,col) weights in {1, xv, xv^2}:
        S    = sum_f w * vec16        (16-vector)
        lin  = sum_f w * (c16 + c17)  (all linear terms)
        ssq  = sum_f w * c18          (sum of squares term)
        out  = lin + 0.5*(|S|^2 - ssq) + bias + const

Device work per core (data-parallel over batch, 2048 samples/core):
  * prefill weight tile, indirect-DMA gather with CCE multiply,
    segmented reduce over the 39 fields, tiny final math.
The gathers (79,872 random 76B rows per core) dominate: memory-bound, as
intended for this problem.
"""

import numpy as np

P = 128            # SBUF partitions
V = 100000         # vocab per field
NF = 39            # fields (13 dense + 26 sparse)
ND = 13            # dense fields
E = 16             # embedding dim
WROW = 19          # unified table row width (floats)
B = 16384          # batch
NCORES = 8
BS = B // NCORES   # 2048 samples per core
SLOTS = BS // P    # 16 samples per partition
CHUNK = 4          # sample-slots per gather chunk
BN_EPS = 1e-5

_prog_cache = {}


def build_program(v=V, nf=NF, nd=ND, w=WROW, slots=SLOTS, chunk=CHUNK):
    """Build (and compile) the per-core Bass program. SPMD: all cores run it."""
    key = (v, nf, nd, w, slots, chunk)
    if key in _prog_cache:
        return _prog_cache[key]

    import concourse.bass as bass
    import concourse.tile as tile
    from concourse import bacc, mybir

    fp = mybir.dt.float32
    nc = bacc.Bacc("TRN2", target_bir_lowering=False, debug=False)

    tab = nc.dram_tensor("tab", [nf * v, w], fp, kind="ExternalInput")
    idx_d = nc.dram_tensor("idx", [P, slots * nf], mybir.dt.int32,
                           kind="ExternalInput")
    xv_d = nc.dram_tensor("xv", [P, slots * nf], fp, kind="ExternalInput")
    bias_d = nc.dram_tensor("biasc", [P, slots], fp, kind="ExternalInput")
    out_d = nc.dram_tensor("out", [P, slots], fp, kind="ExternalOutput")

    ns = nf - nd  # sparse fields
    X = mybir.AxisListType.X
    add = mybir.AluOpType.add
    mult = mybir.AluOpType.mult

    with tile.TileContext(nc) as tc:
        with (
            tc.tile_pool(name="io", bufs=1) as io,
            tc.tile_pool(name="wt", bufs=3) as wtp,
            tc.tile_pool(name="sm", bufs=3) as sm,
        ):
            idx_t = io.tile([P, slots * nf], mybir.dt.int32)
            nc.sync.dma_start(out=idx_t[:], in_=idx_d.ap())
            xv_t = io.tile([P, slots * nf], fp)
            nc.sync.dma_start(out=xv_t[:], in_=xv_d.ap())
            bias_t = io.tile([P, slots], fp)
            nc.sync.dma_start(out=bias_t[:], in_=bias_d.ap())
            out_t = io.tile([P, slots], fp)

            xv3 = xv_t[:].rearrange("p (j f) -> p j f", f=nf)

            # xv^2 for the sparse fields, all slots at once
            xv2_t = io.tile([P, slots, ns], fp)
            nc.vector.tensor_mul(xv2_t[:], xv3[:, :, nd:nf], xv3[:, :, nd:nf])

            for j0 in range(0, slots, chunk):
                c = min(chunk, slots - j0)
                wt = wtp.tile([P, chunk, nf, w], fp, tag="wt")
                xvc = xv3[:, j0:j0 + c, :]
                # --- gather rows: one [P,1]-offset indirect DMA per (slot,field)
                # (multi-offset indirect DMA mis-executes on HW; this is the
                # known-good pattern: one offset per partition)
                for jj in range(c):
                    for f in range(nf):
                        col = (j0 + jj) * nf + f
                        nc.gpsimd.indirect_dma_start(
                            out=wt[:, jj, f, :],
                            out_offset=None,
                            in_=tab.ap(),
                            in_offset=bass.IndirectOffsetOnAxis(
                                ap=idx_t[:, col:col + 1], axis=0),
                        )
                # --- apply the {1, xv, xv^2} weights in place ---
                # sparse fields: cols 0..17 *= xv
                nc.vector.tensor_mul(
                    wt[:, :c, nd:nf, 0:18], wt[:, :c, nd:nf, 0:18],
                    xvc[:, :, nd:nf].unsqueeze(3).to_broadcast([P, c, ns, 18]))
                # sparse fields: col 18 *= xv^2
                nc.vector.tensor_mul(wt[:, :c, nd:nf, 18], wt[:, :c, nd:nf, 18],
                                     xv2_t[:, j0:j0 + c, :])
                # dense fields: col 17 *= xv
                nc.vector.tensor_mul(wt[:, :c, 0:nd, 17], wt[:, :c, 0:nd, 17],
                                     xvc[:, :, 0:nd])
                prod_view = wt[:, :c]

                # --- segmented reduce over the nf fields ---
                red = sm.tile([P, chunk, w], fp, tag="red")
                nc.vector.tensor_reduce(red[:, :c],
                                        prod_view.transpose([0, 1, 3, 2]),
                                        axis=X, op=add)
                # --- 0.5*(|S|^2 - ssq) + lin + bias ---
                sq = sm.tile([P, chunk, E], fp, tag="sq")
                nc.vector.tensor_mul(sq[:, :c], red[:, :c, 0:E], red[:, :c, 0:E])
                ssum = sm.tile([P, chunk], fp, tag="ssum")
                nc.vector.tensor_reduce(ssum[:, :c], sq[:, :c], axis=X, op=add)
                t0 = sm.tile([P, chunk], fp, tag="t0")
                nc.vector.tensor_add(t0[:, :c], red[:, :c, 16], red[:, :c, 17])
                t1 = sm.tile([P, chunk], fp, tag="t1")
                nc.vector.tensor_sub(t1[:, :c], ssum[:, :c], red[:, :c, 18])
                t2 = sm.tile([P, chunk], fp, tag="t2")
                nc.vector.tensor_add(t2[:, :c], t0[:, :c],
                                     bias_t[:, j0:j0 + c])
                nc.vector.scalar_tensor_tensor(
                    out=out_t[:, j0:j0 + c], in0=t1[:, :c], scalar=0.5,
                    in1=t2[:, :c], op0=mult, op1=add)

            nc.sync.dma_start(out=out_d.ap(), in_=out_t[:])

    nc.compile()
    _prog_cache[key] = nc
    return nc


def build_table(W1, b1, emb1, W2, b2, emb2, Lw1, Lb1, g1, be1, Lw2, Lb2, g2,
                be2, v=V):
    """Host-side weight preprocessing: unified table + folded constant."""
    inv = np.float32(1.0 / np.sqrt(1.0 + BN_EPS))
    v2 = (Lw2 @ (inv * g2)).astype(np.float32)            # (512,)
    c2 = float(np.sum(Lb2 * inv * g2 + be2))
    u1 = (inv * g1 * v2).astype(np.float32)               # (512,)
    w_deep = (Lw1 @ u1).astype(np.float32)                # (624,)
    c_deep = float(Lb1 @ u1 + be1 @ v2) + c2
    wd = w_deep.reshape(NF, E)

    tab = np.empty((NF, v, WROW), dtype=np.float32)
    vv = np.arange(v, dtype=np.float32)
    for f in range(ND):
        vecs = vv[:, None] * W2[f][None, :] + b2[f][None, :]
        tab[f, :, :E] = vecs
        tab[f, :, 16] = vecs @ wd[f]
        tab[f, :, 17] = vv * W1[f].sum() + b1[f].sum()
        tab[f, :, 18] = np.einsum('ve,ve->v', vecs, vecs)
    for s in range(NF - ND):
        f = ND + s
        vecs = emb2[s]
        tab[f, :, :E] = vecs
        tab[f, :, 16] = emb1[s].sum(1) + vecs @ wd[f]
        tab[f, :, 17] = 0.0
        tab[f, :, 18] = np.einsum('ve,ve->v', vecs, vecs)
    return tab.reshape(NF * v, WROW), np.float32(c_deep)


def kernel(Xi, Xv, W1, b1, emb1, W2, b2, emb2,
           Lw1, Lb1, g1, be1, Lw2, Lb2, g2, be2, bias):
    import os
    Xi = np.asarray(Xi)
    Xv = np.asarray(Xv, dtype=np.float32)
    tab, c_deep = build_table(
        np.asarray(W1, np.float32), np.asarray(b1, np.float32),
        np.asarray(emb1, np.float32), np.asarray(W2, np.float32),
        np.asarray(b2, np.float32), np.asarray(emb2, np.float32),
        np.asarray(Lw1, np.float32), np.asarray(Lb1, np.float32),
        np.asarray(g1, np.float32), np.asarray(be1, np.float32),
        np.asarray(Lw2, np.float32), np.asarray(Lb2, np.float32),
        np.asarray(g2, np.float32), np.asarray(be2, np.float32))

    flat_idx = (Xi.reshape(B, NF).astype(np.int64)
                + (np.arange(NF, dtype=np.int64) * V)[None, :])
    flat_idx = flat_idx.astype(np.int32)
    biasc = (np.asarray(bias, np.float32) + c_deep).astype(np.float32)

    nc = build_program()

    in_maps = []
    for c in range(NCORES):
        sl = slice(c * BS, (c + 1) * BS)
        in_maps.append({
            "tab": tab,
            "idx": np.ascontiguousarray(flat_idx[sl].reshape(P, SLOTS * NF)),
            "xv": np.ascontiguousarray(Xv[sl].reshape(P, SLOTS * NF)),
            "biasc": np.ascontiguousarray(biasc[sl].reshape(P, SLOTS)),
        })

    from concourse.bass_utils import run_bass_kernel_spmd
    trace = bool(int(os.environ.get("KERNEL_TRACE", "0")))
    res = run_bass_kernel_spmd(nc, in_maps, core_ids=list(range(NCORES)),
                               trace=trace)
    if trace:
        kernel.last_results = res
    out = np.concatenate([r["out"].reshape(BS) for r in res.results])
    return out.astype(np.float32)


# revision 8
# speedup vs baseline: 1.4679x; 1.4679x over previous
"""Trainium2 Bass kernel for nn_DeepFM_27169963114981.

Strategy
--------
The reference DeepFM (eval mode, affine BN, no nonlinearity in the deep MLP)
collapses algebraically:

  * the deep MLP output-sum is LINEAR in its input -> fold into a 624-vector
    w_deep, then fold per-field slices of it into the embedding tables.
  * each sparse field becomes one unified table row of 18 floats:
        row = [ emb2_vec16 | c16 | c17 ]
    with c16 = sum(emb1_row) + emb2_row . wd_field   (all linear terms)
         c17 = sum(emb2_row^2)                        (sum-of-squares term)
    weighted on device by {xv (cols 0..16), xv^2 (col 17)}.
  * dense fields (integer feature value v, scalar xv) are polynomials in v:
    computed on DVE from small replicated constant tables.

Per sample: S = sum_f w*vec16 (incl. dense affine part),
  out = lin + 0.5*(|S|^2 - ssq) + bias + const.

Device work per core (data-parallel over batch, 2048 samples/core):
  416 indirect-DMA gathers of 128x72B rows (one offset per partition --
  the only HW-correct indirect DMA form), broadcast DVE weighting, strided
  segmented reduce over fields, small dense-field polynomial path, tiny tail.
"""

import numpy as np

P = 128            # SBUF partitions
V = 100000         # vocab per field
NF = 39            # total fields
ND = 13            # dense fields
NS = 26            # sparse fields
E = 16             # embedding dim
WROW = 18          # unified sparse table row width (floats)
B = 16384          # batch
NCORES = 8
BS = B // NCORES   # 2048 samples per core
SLOTS = BS // P    # 16 samples per partition
CHUNK = 4          # sample-slots per chunk
BN_EPS = 1e-5
NCONST = 208 + 5 * 13 + 16   # 289

_prog_cache = {}


def build_program(v=V, ns=NS, nd=ND, w=WROW, slots=SLOTS, chunk=CHUNK):
    """Build (and compile) the per-core Bass program. SPMD: all cores run it."""
    key = (v, ns, nd, w, slots, chunk)
    if key in _prog_cache:
        return _prog_cache[key]

    import concourse.bass as bass
    import concourse.tile as tile
    from concourse import bacc, mybir

    fp = mybir.dt.float32
    nc = bacc.Bacc("TRN2", target_bir_lowering=False, debug=False)

    tab = nc.dram_tensor("tab", [ns * v, w], fp, kind="ExternalInput")
    idx_d = nc.dram_tensor("idx", [P, slots * ns], mybir.dt.int32,
                           kind="ExternalInput")
    xv_d = nc.dram_tensor("xv", [P, slots * (nd + ns)], fp, kind="ExternalInput")
    xif_d = nc.dram_tensor("xif", [P, slots * nd], fp, kind="ExternalInput")
    cst_d = nc.dram_tensor("cst", [P, NCONST], fp, kind="ExternalInput")
    bias_d = nc.dram_tensor("biasc", [P, slots], fp, kind="ExternalInput")
    out_d = nc.dram_tensor("out", [P, slots], fp, kind="ExternalOutput")

    nf = nd + ns
    X = mybir.AxisListType.X
    add = mybir.AluOpType.add
    mult = mybir.AluOpType.mult

    with tile.TileContext(nc) as tc:
        with (
            tc.tile_pool(name="io", bufs=1) as io,
            tc.tile_pool(name="wt", bufs=3) as wtp,
            tc.tile_pool(name="sm", bufs=3) as sm,
        ):
            idx_t = io.tile([P, slots * ns], mybir.dt.int32)
            nc.sync.dma_start(out=idx_t[:], in_=idx_d.ap())
            xv_t = io.tile([P, slots * nf], fp)
            nc.sync.dma_start(out=xv_t[:], in_=xv_d.ap())
            xif_t = io.tile([P, slots * nd], fp)
            nc.sync.dma_start(out=xif_t[:], in_=xif_d.ap())
            cst_t = io.tile([P, NCONST], fp)
            nc.sync.dma_start(out=cst_t[:], in_=cst_d.ap())
            bias_t = io.tile([P, slots], fp)
            nc.sync.dma_start(out=bias_t[:], in_=bias_d.ap())
            out_t = io.tile([P, slots], fp)

            xv3 = xv_t[:].rearrange("p (j f) -> p j f", f=nf)
            xif3 = xif_t[:].rearrange("p (j f) -> p j f", f=nd)
            cst = cst_t[:]
            W2r = cst[:, 0:208].rearrange("p (f e) -> p f e", f=nd)
            d1r = cst[:, 208:221]
            sW1r = cst[:, 221:234]
            sb1r = cst[:, 234:247]
            alr = cst[:, 247:260]
            ber = cst[:, 260:273]
            b2sr = cst[:, 273:289]

            # xv^2 for the sparse fields, all slots at once
            xv2_t = io.tile([P, slots, ns], fp)
            nc.vector.tensor_mul(xv2_t[:], xv3[:, :, nd:nf], xv3[:, :, nd:nf])

            for j0 in range(0, slots, chunk):
                c = min(chunk, slots - j0)
                wt = wtp.tile([P, chunk, ns, w], fp, tag="wt")
                xvc = xv3[:, j0:j0 + c, nd:nf]            # sparse xv [P,c,ns]
                xic = xif3[:, j0:j0 + c, :]               # dense xi  [P,c,nd]
                # --- gather sparse rows: one [P,1]-offset DMA per (slot,field)
                for jj in range(c):
                    for f in range(ns):
                        col = (j0 + jj) * ns + f
                        nc.gpsimd.indirect_dma_start(
                            out=wt[:, jj, f, :],
                            out_offset=None,
                            in_=tab.ap(),
                            in_offset=bass.IndirectOffsetOnAxis(
                                ap=idx_t[:, col:col + 1], axis=0),
                        )
                # --- apply the {xv, xv^2} weights in place ---
                nc.vector.tensor_mul(
                    wt[:, :c, :, 0:17], wt[:, :c, :, 0:17],
                    xvc.unsqueeze(3).to_broadcast([P, c, ns, 17]))
                nc.vector.tensor_mul(wt[:, :c, :, 17], wt[:, :c, :, 17],
                                     xv2_t[:, j0:j0 + c, :])

                # --- segmented reduce over the ns sparse fields ---
                red = sm.tile([P, chunk, w], fp, tag="red")
                nc.vector.tensor_reduce(red[:, :c],
                                        wt[:, :c].transpose([0, 1, 3, 2]),
                                        axis=X, op=add)

                # --- dense fields on DVE ---
                tmp16 = sm.tile([P, chunk, nd, E], fp, tag="tmp16")
                nc.vector.tensor_mul(
                    tmp16[:, :c],
                    xic.unsqueeze(3).to_broadcast([P, c, nd, E]),
                    W2r.unsqueeze(1).to_broadcast([P, c, nd, E]))
                S = sm.tile([P, chunk, E], fp, tag="S")
                nc.vector.tensor_reduce(S[:, :c],
                                        tmp16[:, :c].transpose([0, 1, 3, 2]),
                                        axis=X, op=add)
                # S += sparse vec + b2sum
                nc.vector.tensor_add(S[:, :c], S[:, :c], red[:, :c, 0:E])
                nc.vector.tensor_add(S[:, :c], S[:, :c],
                                     b2sr.unsqueeze(1).to_broadcast([P, c, E]))
                # dense scalars: a = xv*(xi*sW1 + sb1) + xi*d1 ; q = (al*xi+be)*xi
                a_t = sm.tile([P, chunk, nd], fp, tag="a")
                nc.vector.tensor_mul(a_t[:, :c], xic,
                                     sW1r.unsqueeze(1).to_broadcast([P, c, nd]))
                nc.vector.tensor_add(a_t[:, :c], a_t[:, :c],
                                     sb1r.unsqueeze(1).to_broadcast([P, c, nd]))
                nc.vector.tensor_mul(a_t[:, :c], a_t[:, :c],
                                     xv3[:, j0:j0 + c, 0:nd])
                d_t = sm.tile([P, chunk, nd], fp, tag="d")
                nc.vector.tensor_mul(d_t[:, :c], xic,
                                     d1r.unsqueeze(1).to_broadcast([P, c, nd]))
                nc.vector.tensor_add(a_t[:, :c], a_t[:, :c], d_t[:, :c])
                q_t = sm.tile([P, chunk, nd], fp, tag="q")
                nc.vector.tensor_mul(q_t[:, :c], xic,
                                     alr.unsqueeze(1).to_broadcast([P, c, nd]))
                nc.vector.tensor_add(q_t[:, :c], q_t[:, :c],
                                     ber.unsqueeze(1).to_broadcast([P, c, nd]))
                nc.vector.tensor_mul(q_t[:, :c], q_t[:, :c], xic)
                lin_d = sm.tile([P, chunk], fp, tag="lin_d")
                nc.vector.tensor_reduce(lin_d[:, :c], a_t[:, :c], axis=X, op=add)
                q_d = sm.tile([P, chunk], fp, tag="q_d")
                nc.vector.tensor_reduce(q_d[:, :c], q_t[:, :c], axis=X, op=add)

                # --- 0.5*(|S|^2 - ssq) + lin + bias ---
                sq = sm.tile([P, chunk, E], fp, tag="sq")
                nc.vector.tensor_mul(sq[:, :c], S[:, :c], S[:, :c])
                ssum = sm.tile([P, chunk], fp, tag="ssum")
                nc.vector.tensor_reduce(ssum[:, :c], sq[:, :c], axis=X, op=add)
                t0 = sm.tile([P, chunk], fp, tag="t0")
                nc.vector.tensor_add(t0[:, :c], red[:, :c, 16], lin_d[:, :c])
                nc.vector.tensor_add(t0[:, :c], t0[:, :c], bias_t[:, j0:j0 + c])
                t1 = sm.tile([P, chunk], fp, tag="t1")
                nc.vector.tensor_add(t1[:, :c], red[:, :c, 17], q_d[:, :c])
                t2 = sm.tile([P, chunk], fp, tag="t2")
                nc.vector.tensor_sub(t2[:, :c], ssum[:, :c], t1[:, :c])
                nc.vector.scalar_tensor_tensor(
                    out=out_t[:, j0:j0 + c], in0=t2[:, :c], scalar=0.5,
                    in1=t0[:, :c], op0=mult, op1=add)

            nc.sync.dma_start(out=out_d.ap(), in_=out_t[:])

    nc.compile()
    _prog_cache[key] = nc
    return nc


def build_host_data(W1, b1, emb1, W2, b2, emb2, Lw1, Lb1, g1, be1,
                    Lw2, Lb2, g2, be2, v=V):
    """Host-side weight preprocessing: sparse table, dense consts, bias const."""
    inv = np.float32(1.0 / np.sqrt(1.0 + BN_EPS))
    v2 = (Lw2 @ (inv * g2)).astype(np.float32)
    c2 = float(np.sum(Lb2 * inv * g2 + be2))
    u1 = (inv * g1 * v2).astype(np.float32)
    w_deep = (Lw1 @ u1).astype(np.float32)
    c_deep = float(Lb1 @ u1 + be1 @ v2) + c2
    wd = w_deep.reshape(NF, E)

    tab = np.empty((NS, v, WROW), dtype=np.float32)
    for s in range(NS):
        vecs = emb2[s]
        tab[s, :, :E] = vecs
        tab[s, :, 16] = emb1[s].sum(1) + vecs @ wd[ND + s]
        tab[s, :, 17] = np.einsum('ve,ve->v', vecs, vecs)

    cst = np.zeros(NCONST, np.float32)
    cst[0:208] = W2.reshape(-1)
    cst[208:221] = np.einsum('fe,fe->f', W2, wd[:ND])
    cst[221:234] = W1.sum(1)
    cst[234:247] = b1.sum(1)
    cst[247:260] = (W2 * W2).sum(1)
    cst[260:273] = 2.0 * (W2 * b2).sum(1)
    cst[273:289] = b2.sum(0)

    bias_const = np.float32(
        c_deep + float(np.einsum('fe,fe->', b2, wd[:ND]))
        - 0.5 * float((b2 * b2).sum()))
    return tab.reshape(NS * v, WROW), cst, bias_const


def kernel(Xi, Xv, W1, b1, emb1, W2, b2, emb2,
           Lw1, Lb1, g1, be1, Lw2, Lb2, g2, be2, bias):
    import os
    Xi = np.asarray(Xi).reshape(B, NF)
    Xv = np.asarray(Xv, dtype=np.float32)
    tab, cst, bias_const = build_host_data(
        np.asarray(W1, np.float32), np.asarray(b1, np.float32),
        np.asarray(emb1, np.float32), np.asarray(W2, np.float32),
        np.asarray(b2, np.float32), np.asarray(emb2, np.float32),
        np.asarray(Lw1, np.float32), np.asarray(Lb1, np.float32),
        np.asarray(g1, np.float32), np.asarray(be1, np.float32),
        np.asarray(Lw2, np.float32), np.asarray(Lb2, np.float32),
        np.asarray(g2, np.float32), np.asarray(be2, np.float32))

    flat_idx = (Xi[:, ND:].astype(np.int64)
                + (np.arange(NS, dtype=np.int64) * V)[None, :]).astype(np.int32)
    xif = Xi[:, :ND].astype(np.float32)
    biasc = (np.asarray(bias, np.float32) + bias_const).astype(np.float32)
    cst_tile = np.tile(cst[None, :], (P, 1))

    nc = build_program()

    in_maps = []
    for c in range(NCORES):
        sl = slice(c * BS, (c + 1) * BS)
        in_maps.append({
            "tab": tab,
            "idx": np.ascontiguousarray(flat_idx[sl].reshape(P, SLOTS * NS)),
            "xv": np.ascontiguousarray(Xv[sl].reshape(P, SLOTS * NF)),
            "xif": np.ascontiguousarray(xif[sl].reshape(P, SLOTS * ND)),
            "cst": cst_tile,
            "biasc": np.ascontiguousarray(biasc[sl].reshape(P, SLOTS)),
        })

    from concourse.bass_utils import run_bass_kernel_spmd
    trace = bool(int(os.environ.get("KERNEL_TRACE", "0")))
    res = run_bass_kernel_spmd(nc, in_maps, core_ids=list(range(NCORES)),
                               trace=trace)
    if trace:
        kernel.last_results = res
    out = np.concatenate([r["out"].reshape(BS) for r in res.results])
    return out.astype(np.float32)
